# revision 5
# baseline (speedup 1.0000x reference)
"""Trainium2 Bass kernel for AttentionAggregation (segment_reduce).

Math (reference):
    v      = tanh(x @ Vw.T + Vb)                 [N, H]
    alpha  = exp(v @ ww.T + wb)                  [N, 1]
    s_b    = segsum(alpha)                       [B, 1]
    y_b    = segsum(alpha * x)                   [B, E]   (unnormalized)
    bagsum = y_b / s_b                           [B, E]
    out    = softmax(bagsum @ dw.T + db)         [B, 2]

Sharding: contiguous row ranges split at bag boundaries, 64 whole bags per
core. Each core computes final probs for its 64 bags; host concatenates.

Kernel structure per 512-row supertile (one shared SPMD program, 8 cores):
  - SWDGE cast-DMA loads x fp32->bf16 in natural layout [128, blocks, 512]
  - PE transposes build xT (E on partitions) for the V/w projections
  - PE matmuls: vT = VwT.T @ xT (tanh on ACT), score = wwT.T @ tanh(vT)
  - alpha = exp(score) (ACT), transposed back to natural via tiny PE transposes
  - DVE builds per-block selection matrices S[i,w] = alpha_i * (idxlocal_i == w)
    over a static bag window per supertile (sorted indices => narrow window,
    verified on host at build time)
  - PE segment-matmul accumulates acc[slot, :] += S.T @ x_block and
    acc_s[slot] += S.T @ 1 into a persistent PSUM bank across the whole run
  - epilogue: bagsum = acc/acc_s, logits = bagsum @ dw.T + db, softmax -> [64, 2]
"""

import sys

for _p in ("/opt/trn_rl_repo", "/root/.axon_site/_ro/trn_rl_repo"):
    if _p not in sys.path:
        sys.path.append(_p)

import numpy as np
import ml_dtypes

import concourse.bass as bass
import concourse.bacc as bacc
import concourse.tile as tile
from concourse import mybir
from concourse.bass_utils import run_bass_kernel_spmd

F32 = mybir.dt.float32
BF16 = mybir.dt.bfloat16
BFNP = ml_dtypes.bfloat16

N, E, H, B = 262144, 512, 256, 512
NCORES = 8
BAGS_PER_CORE = B // NCORES  # 64
P = 128
RPT = 512          # rows per supertile
BPS = 4            # 128-row blocks per supertile
G = 2              # supertiles per DMA batch (2MiB fp32 reads)

AF = mybir.ActivationFunctionType
OP = mybir.AluOpType

_cache = {}


def _build_program(n_st, MD, MU):
    """Build the shared SPMD Bass program. Static over (n_st, MD, MU)."""
    # One full-width selection window: slot = local_bag + MD, identical
    # psum partition base for every accumulating matmul (PSUM zero-region
    # semantics clobber overlaps when the out base shifts between matmuls).
    SW = 128
    PACC = 128
    assert MD + BAGS_PER_CORE + MU <= SW and n_st % G == 0

    nc = bacc.Bacc("TRN2", target_bir_lowering=False)

    x_d = nc.declare_dram_parameter("x", [n_st * RPT, E], F32, isOutput=False)
    idx_d = nc.declare_dram_parameter("idxl", [n_st, P, BPS], F32, isOutput=False)
    vwT_d = nc.declare_dram_parameter("vwT", [P, 4 * H], BF16, isOutput=False)
    wwT_d = nc.declare_dram_parameter("wwT", [P, 2], BF16, isOutput=False)
    vbT_d = nc.declare_dram_parameter("vbT", [P, 2], F32, isOutput=False)
    wb_d = nc.declare_dram_parameter("wb", [1, 1], F32, isOutput=False)
    dwT_d = nc.declare_dram_parameter("dwT", [P, 8], F32, isOutput=False)
    db_d = nc.declare_dram_parameter("db", [2, 1], F32, isOutput=False)
    idb_d = nc.declare_dram_parameter("idb", [P, P], BF16, isOutput=False)
    idf_d = nc.declare_dram_parameter("idf", [P, P], F32, isOutput=False)
    onef_d = nc.declare_dram_parameter("onef", [1, 1], F32, isOutput=False)
    iota_d = nc.declare_dram_parameter("iota", [P, SW], F32, isOutput=False)
    ones_d = nc.declare_dram_parameter("onesb", [P, 1], BF16, isOutput=False)
    zros_d = nc.declare_dram_parameter("zrosb", [P, PACC], BF16, isOutput=False)
    c512_d = nc.declare_dram_parameter("c512b", [P, RPT], BF16, isOutput=False)
    out_d = nc.declare_dram_parameter("out", [BAGS_PER_CORE, 2], F32, isOutput=True)

    with tile.TileContext(nc) as tc:
        with (
            tc.tile_pool(name="consts", bufs=1) as cpool,
            tc.tile_pool(name="xin", bufs=2) as xpool,
            tc.tile_pool(name="xT", bufs=2) as xTpool,
            tc.tile_pool(name="vs", bufs=2) as vpool,
            tc.tile_pool(name="small", bufs=2) as smpool,
            tc.tile_pool(name="sel", bufs=4) as spool,
            tc.tile_pool(name="epi", bufs=1) as epool,
            tc.tile_pool(name="pacc", bufs=1, space="PSUM") as pacc,
            tc.tile_pool(name="pxT", bufs=2, space="PSUM") as pxT,
            tc.tile_pool(name="pv", bufs=2, space="PSUM") as pv,
            tc.tile_pool(name="pa", bufs=1, space="PSUM") as pa,
            tc.tile_pool(name="pan", bufs=1, space="PSUM") as pan,
        ):
            # ---- load constants into SBUF
            vwTs = cpool.tile([P, 4 * H], BF16)
            nc.sync.dma_start(out=vwTs, in_=vwT_d[:, :])
            wwTs = cpool.tile([P, 2], BF16)
            nc.sync.dma_start(out=wwTs, in_=wwT_d[:, :])
            vbTs = cpool.tile([P, 2], F32)
            nc.sync.dma_start(out=vbTs, in_=vbT_d[:, :])
            wbs = cpool.tile([1, 1], F32)
            nc.sync.dma_start(out=wbs, in_=wb_d[:, :])
            dwTs = cpool.tile([P, 8], F32)
            nc.sync.dma_start(out=dwTs, in_=dwT_d[:, :])
            dbs = cpool.tile([2, 1], F32)
            nc.sync.dma_start(out=dbs, in_=db_d[:, :])
            idb = cpool.tile([P, P], BF16)
            nc.sync.dma_start(out=idb, in_=idb_d[:, :])
            idf = cpool.tile([P, P], F32)
            nc.sync.dma_start(out=idf, in_=idf_d[:, :])
            onef = cpool.tile([1, 1], F32)
            nc.sync.dma_start(out=onef, in_=onef_d[:, :])
            iota = cpool.tile([P, SW], F32)
            nc.sync.dma_start(out=iota, in_=iota_d[:, :])
            onesb = cpool.tile([P, 1], BF16)
            nc.sync.dma_start(out=onesb, in_=ones_d[:, :])
            zrosb = cpool.tile([P, PACC], BF16)
            nc.sync.dma_start(out=zrosb, in_=zros_d[:, :])
            c512b = cpool.tile([P, RPT], BF16)
            nc.sync.dma_start(out=c512b, in_=c512_d[:, :])

            # ---- persistent PSUM accumulators, cleared via zero-weight matmuls
            acc = pacc.tile([PACC, E], F32)
            acc_s = pacc.tile([PACC, 1], F32)
            nc.tensor.matmul(acc[:, :], lhsT=zrosb[:, :], rhs=c512b[:, :],
                             start=True, stop=False)
            nc.tensor.matmul(acc_s[:, :], lhsT=zrosb[:, :], rhs=onesb[:, :],
                             start=True, stop=False)

            xview = x_d.rearrange("(g r p) e -> g p r e", p=P, r=G * BPS)

            xt = None
            for t in range(n_st):
                gb, r = divmod(t, G)
                if r == 0:
                    xt = xpool.tile([P, G * BPS, E], BF16, tag="xt")
                    # SWDGE cast-DMA: fp32 HBM -> bf16 SBUF
                    nc.gpsimd.dma_start(out=xt[:, :, :], in_=xview[gb])

                idxt = smpool.tile([P, BPS], F32, tag="idxt")
                nc.sync.dma_start(out=idxt, in_=idx_d[t])
                idxw = smpool.tile([P, BPS], F32, tag="idxw")
                nc.vector.tensor_scalar_add(idxw, idxt, float(MD))

                # xT[ec] = x supertile block-transposed: [128e, 512rows]
                xTs = xTpool.tile([P, 4, RPT], BF16, tag="xTs")
                for ec in range(4):
                    xTp = pxT.tile([P, RPT], BF16, tag="xTp")
                    for a in range(BPS):
                        nc.tensor.transpose(
                            xTp[:, a * P:(a + 1) * P],
                            xt[:, r * BPS + a, ec * P:(ec + 1) * P],
                            idb[:, :])
                    if ec % 2 == 0:
                        nc.scalar.copy(xTs[:, ec, :], xTp[:, :])
                    else:
                        nc.vector.tensor_copy(xTs[:, ec, :], xTp[:, :])

                # vT = tanh(VwT.T @ xT + Vb) per H-chunk
                vs = vpool.tile([P, 2, RPT], BF16, tag="vs")
                for hc in range(2):
                    vp = pv.tile([P, RPT], F32, tag="vp")
                    for ec in range(4):
                        nc.tensor.matmul(
                            vp[:, :],
                            lhsT=vwTs[:, ec * H + hc * P: ec * H + (hc + 1) * P],
                            rhs=xTs[:, ec, :],
                            start=(ec == 0), stop=(ec == 3))
                    nc.scalar.activation(vs[:, hc, :], vp[:, :], AF.Tanh,
                                         bias=vbTs[:, hc:hc + 1])

                # alpha^T = exp(ww . v + wb)  [1, 512]
                aps = pa.tile([1, RPT], F32, tag="aps")
                for hc in range(2):
                    nc.tensor.matmul(aps[:, :], lhsT=wwTs[:, hc:hc + 1],
                                     rhs=vs[:, hc, :],
                                     start=(hc == 0), stop=(hc == 1))
                aex = smpool.tile([1, RPT], F32, tag="aex")
                nc.scalar.activation(aex, aps[:, :], AF.Exp, bias=wbs[:, 0:1])

                # alpha back to natural layout [128, 4] via tiny PE transposes
                anp = pan.tile([P, BPS], F32, tag="anp")
                for a in range(BPS):
                    nc.tensor.transpose(anp[:, a:a + 1],
                                        aex[:, a * P:(a + 1) * P], onef[:, :])
                ana = smpool.tile([P, BPS], F32, tag="ana")
                nc.vector.tensor_copy(ana, anp[:, :])

                # selection matmuls: acc[t:t+W] += S.T @ x_block, acc_s += S.T @ 1
                for a in range(BPS):
                    s0 = spool.tile([P, SW], F32, tag="s0")
                    nc.vector.tensor_tensor(
                        out=s0, in0=iota[:, :],
                        in1=idxw[:, a:a + 1].to_broadcast([P, SW]),
                        op=OP.is_equal)
                    sa = spool.tile([P, SW], BF16, tag="sa")
                    nc.vector.tensor_tensor(
                        out=sa, in0=s0,
                        in1=ana[:, a:a + 1].to_broadcast([P, SW]),
                        op=OP.mult)
                    nc.tensor.matmul(acc[:, :], lhsT=sa,
                                     rhs=xt[:, r * BPS + a, :],
                                     start=False, stop=False)
                    nc.tensor.matmul(acc_s[:, :], lhsT=sa,
                                     rhs=onesb[:, :], start=False, stop=False)

            # close the accumulation groups (adds zero)
            nc.tensor.matmul(acc[:, :], lhsT=zrosb[:, :], rhs=c512b[:, :],
                             start=False, stop=True)
            nc.tensor.matmul(acc_s[:, :], lhsT=zrosb[:, :], rhs=onesb[:, :],
                             start=False, stop=True)

            # ---- epilogue: bagsum = acc / acc_s; logits = bagsum @ dw.T + db
            ysb = epool.tile([PACC, E], F32)
            nc.vector.tensor_copy(ysb, acc[:, :])
            ssb = epool.tile([PACC, 1], F32)
            nc.scalar.activation(ssb, acc_s[:, :], AF.Copy, bias=0.0)
            sse = epool.tile([PACC, 1], F32)
            nc.vector.tensor_scalar_add(sse, ssb, 1e-30)
            rs = epool.tile([PACC, 1], F32)
            nc.vector.reciprocal(rs, sse)
            ynorm = epool.tile([PACC, E], F32)
            nc.vector.tensor_tensor(out=ynorm, in0=ysb,
                                    in1=rs.to_broadcast([PACC, E]), op=OP.mult)

            yTp = pxT.tile([P, 4, PACC], F32, tag="xTp")
            for ec in range(4):
                nc.tensor.transpose(yTp[:, ec, :],
                                    ynorm[:, ec * P:(ec + 1) * P],
                                    idf[:PACC, :PACC])
            yTs = epool.tile([P, 4, PACC], F32)
            nc.vector.tensor_copy(yTs, yTp[:, :, :])

            lgp = pv.tile([2, PACC], F32, tag="vp")
            for ec in range(4):
                nc.tensor.matmul(lgp[:, :], lhsT=dwTs[:, 2 * ec:2 * ec + 2],
                                 rhs=yTs[:, ec, :],
                                 start=(ec == 0), stop=(ec == 3))
            lgs = epool.tile([2, PACC], F32)
            nc.vector.tensor_tensor(out=lgs, in0=lgp[:, :],
                                    in1=dbs.to_broadcast([2, PACC]), op=OP.add)

            lgnp = pan.tile([PACC, 2], F32, tag="anp")
            nc.tensor.transpose(lgnp[:, :], lgs[:, :], idf[:2, :2])
            en = epool.tile([PACC, 2], F32)
            nc.scalar.activation(en, lgnp[:, :], AF.Exp)
            s2 = epool.tile([PACC, 1], F32)
            nc.vector.tensor_reduce(s2, en, axis=mybir.AxisListType.X, op=OP.add)
            r2 = epool.tile([PACC, 1], F32)
            nc.vector.reciprocal(r2, s2)
            probs = epool.tile([PACC, 2], F32)
            nc.vector.tensor_tensor(out=probs, in0=en,
                                    in1=r2.to_broadcast([PACC, 2]), op=OP.mult)
            nc.sync.dma_start(out=out_d[:, :], in_=probs[MD:MD + BAGS_PER_CORE, :])

    if not nc.is_finalized():
        nc.finalize()
    return nc


def _prepare(bag_encoding, Vw, Vb, ww, wb, dw, db, batch_indices):
    """Host-side sharding + constant packing. Returns (plan, in_maps)."""
    x = np.ascontiguousarray(np.asarray(bag_encoding, dtype=np.float32))
    idx = np.asarray(batch_indices).astype(np.int64)
    assert x.shape == (N, E) and idx.shape == (N,)

    # bag start offsets (indices are sorted)
    starts = np.searchsorted(idx, np.arange(B + 1), side="left")
    core_lo = starts[np.arange(NCORES) * BAGS_PER_CORE]
    core_hi = starts[np.arange(NCORES) * BAGS_PER_CORE + BAGS_PER_CORE]
    rcounts = core_hi - core_lo
    n_st = int(np.ceil(rcounts.max() / (RPT * G)) * G)
    rpad = n_st * RPT

    # per-core local indices (float32), padding rows get -1e6 (never matches)
    idxls, xslices = [], []
    md_need, mu_need = 0, 0
    for c in range(NCORES):
        lo, hi = int(core_lo[c]), int(core_hi[c])
        il = np.full(rpad, -1.0e6, np.float32)
        il[:hi - lo] = (idx[lo:hi] - c * BAGS_PER_CORE).astype(np.float32)
        # window check: every real row's local bag must be in [t-MD, t+MU]
        st_of_row = np.arange(hi - lo) // RPT
        d = il[:hi - lo] - st_of_row
        md_need = max(md_need, int(np.ceil(-d.min())))
        mu_need = max(mu_need, int(np.ceil(d.max())))
        idxls.append(il)
        xs = np.zeros((rpad, E), np.float32)
        xs[:hi - lo] = x[lo:hi]
        xslices.append(xs)

    MD, MU = md_need + 1, mu_need + 1
    assert MD + 64 + MU <= 128, (MD, MU)

    # shared constants, packed partition-major
    Vw = np.asarray(Vw, np.float32)     # [H, E]
    Vb = np.asarray(Vb, np.float32)     # [H]
    ww = np.asarray(ww, np.float32)     # [1, H]
    wb = np.asarray(wb, np.float32)     # [1]
    dw = np.asarray(dw, np.float32)     # [2, E]
    db = np.asarray(db, np.float32)     # [2]

    vwT = np.zeros((P, 4 * H), np.float32)
    for ec in range(4):
        # lhsT block [K=128e, M=128h] per (ec, hc): vwT[p, ec*H + h] = Vw[h, ec*128+p]
        vwT[:, ec * H:(ec + 1) * H] = Vw[:, ec * P:(ec + 1) * P].T
    wwT = np.stack([ww[0, :P], ww[0, P:]], axis=1)          # [128, 2]
    vbT = np.stack([Vb[:P], Vb[P:]], axis=1)                # [128, 2]
    dwT = np.zeros((P, 8), np.float32)
    for ec in range(4):
        dwT[:, 2 * ec:2 * ec + 2] = dw[:, ec * P:(ec + 1) * P].T
    PACC = 128

    shared = {
        "vwT": vwT.astype(BFNP),
        "wwT": wwT.astype(BFNP),
        "vbT": vbT.astype(np.float32),
        "wb": wb.reshape(1, 1).astype(np.float32),
        "dwT": dwT.astype(np.float32),
        "db": db.reshape(2, 1).astype(np.float32),
        "idb": np.eye(P, dtype=BFNP),
        "idf": np.eye(P, dtype=np.float32),
        "onef": np.ones((1, 1), np.float32),
        "iota": np.tile(np.arange(128, dtype=np.float32), (P, 1)),
        "onesb": np.ones((P, 1), BFNP),
        "zrosb": np.zeros((P, PACC), BFNP),
        "c512b": np.zeros((P, RPT), BFNP),
    }

    in_maps = []
    for c in range(NCORES):
        m = dict(shared)
        m["x"] = xslices[c]
        m["idxl"] = np.ascontiguousarray(
            idxls[c].reshape(n_st, BPS, P).transpose(0, 2, 1))
        in_maps.append(m)
    return (n_st, MD, MU), in_maps


def kernel(bag_encoding, Vw, Vb, ww, wb, dw, db, batch_indices):
    plan, in_maps = _prepare(bag_encoding, Vw, Vb, ww, wb, dw, db,
                             batch_indices)
    if plan not in _cache:
        _cache[plan] = _build_program(*plan)
    nc = _cache[plan]
    res = run_bass_kernel_spmd(nc, in_maps, core_ids=list(range(NCORES)))
    out = np.concatenate([res.results[c]["out"] for c in range(NCORES)], axis=0)
    return out.astype(np.float32)


if __name__ == "__main__":
    sys.path.insert(0, "/root/problem")
    import reference
    inputs = reference.setup_inputs()
    inputs = {k: np.asarray(v) for k, v in inputs.items()}
    got = kernel(**inputs)
    print(got[:4])
